# revision 31
# baseline (speedup 1.0000x reference)
"""Trainium2 Bass kernel for nn_MultiHeadLocalAttention (band-limited attention).

Math: scores are multiplied by a band-count matrix C that is zero outside
|q-k|<=4, then a FULL-row softmax is applied.  Out-of-band entries contribute
exp(0)=1, so with E = exp(C*S) over a 136-wide (128 block + 8 halo) k-window:

  out[q]   = (sum_win E[k,q] vh[k] + Vbg_j) / denom[q]
  denom[q] = sum_win E[k,q] + count_j

where Vbg_j = sum of vh over rows NOT in window j (host-folded through Wv by
linearity) and count_j = 2048 - 136 (phantom-slot compensation makes this
constant even at sequence boundaries).  This makes the O(seq^2) attention an
O(seq*band) computation, exact to fp rounding.

v2: all matmuls in bf16 (fp32 matmul runs as two PE passes and blocks fast
weight load - the fp32 baseline was 100% tensor-engine-bound at 218us),
projections packed to M=128, d_k=64 score matmuls row-tiled in head pairs
(partition bases 0/64).  The concurrent row-tiled pair MUST drain to
different PSUM banks (slot order s=4*par+hp): same-bank concurrent drain
hard-faults real HW while passing CoreSim.

Sharding: batch*seq rows split across 8 cores (512 rows each); each core
computes all 8 heads for its rows.  k/v inputs carry a +-4 halo.  Everything
is traced as one SPMD Bass/Tile program; per-core tensors differ only in data.
"""

import math
import sys
from contextlib import ExitStack

import numpy as np
import ml_dtypes

sys.path.insert(0, "/opt/trn_rl_repo")

import concourse.bass as bass
import concourse.tile as tile
from concourse import bacc, mybir
from concourse.bass_utils import run_bass_kernel_spmd

F32 = mybir.dt.float32
BF16 = mybir.dt.bfloat16
NPBF = ml_dtypes.bfloat16
SEQ, DM, H, DK = 2048, 512, 8, 64
ROWS = 512          # query rows per core
HALO = ROWS + 8     # padded k/v halo columns per core
J = 4               # 128-row query tiles per core
WBAND = 2
BGCNT = float(SEQ - 136)  # background count per q row (constant, see header)


# ----------------------------------------------------------------------------
# host-side helpers
# ----------------------------------------------------------------------------

def _band_count(seq=SEQ, window=WBAND):
    i = np.arange(seq)
    lo = np.clip(i - window, 0, None)
    hi = np.clip(i + window, None, seq - 1)
    lo = np.where(i == 1, 0, lo)
    hi = np.where(i == 1, window + 1, hi)
    lo = np.where(i == seq - 2, seq - window - 2, lo)
    hi = np.where(i == seq - 2, seq - 1, hi)
    a = np.arange(seq)[None, :]
    M = ((a >= lo[:, None]) & (a <= hi[:, None])).astype(np.float32)
    return M.T @ M


def _c_tiles(R0, C):
    """CA [J,128,128], CB [J,8,128] with C[k,q]/sqrt(dk) (C symmetric)."""
    CA = np.zeros((J, 128, 128), np.float32)
    CB = np.zeros((J, 8, 128), np.float32)
    for j in range(J):
        qg = R0 + 128 * j + np.arange(128)
        kgA = R0 - 4 + 128 * j + np.arange(128)
        kgB = R0 - 4 + 128 * j + 128 + np.arange(8)
        mA = (kgA >= 0) & (kgA < SEQ)
        mB = (kgB >= 0) & (kgB < SEQ)
        CA[j][mA, :] = C[np.ix_(kgA[mA], qg)]
        CB[j][mB, :] = C[np.ix_(kgB[mB], qg)]
    return CA, CB


# ----------------------------------------------------------------------------
# device program
# ----------------------------------------------------------------------------

def _build_program(with_bias, upto=4):
    nc = bacc.Bacc()
    E_IN = dict(kind="ExternalInput")
    # inputs coalesced into 3 packed transfers: each dma_start costs ~0.8us
    # of sync-engine issue time, so 15 transfers was pure head latency
    p1_d = nc.dram_tensor("p1", [128, 4096], BF16, **E_IN)   # Wq | qT
    p2_d = nc.dram_tensor("p2", [128, 8256], BF16, **E_IN)   # Wk|kT|Wv|vT
    p3_d = nc.dram_tensor("p3", [128, 2720], BF16, **E_IN)   # Wo|CA|CBs|id
    vs65_d = nc.dram_tensor("vs65", [1, J * H * 66], BF16, **E_IN)
    if with_bias:
        bq_d = nc.dram_tensor("bq", [1, DM], BF16, **E_IN)
        bk_d = nc.dram_tensor("bk", [1, DM], BF16, **E_IN)
        bv_d = nc.dram_tensor("bv", [1, DM], BF16, **E_IN)
    out_d = nc.dram_tensor("out", [ROWS, DM], F32, kind="ExternalOutput")

    MULT = mybir.AluOpType.mult
    EXP = mybir.ActivationFunctionType.Exp

    with tile.TileContext(nc) as tc, ExitStack() as ctx:
        sing = ctx.enter_context(tc.tile_pool(name="sing", bufs=1))

        # --- load everything to SBUF in 4 packed transfers ---
        sb_p1 = sing.tile([128, 4096], BF16)
        nc.sync.dma_start(sb_p1, p1_d[:])
        sb_p2 = sing.tile([128, 8256], BF16)
        nc.sync.dma_start(sb_p2, p2_d[:])
        sb_p3 = sing.tile([128, 2720], BF16)
        nc.sync.dma_start(sb_p3, p3_d[:])
        sb_vs65 = sing.tile([1, J, H, 66], BF16)
        nc.sync.dma_start(sb_vs65, vs65_d[:].rearrange("p (j h e) -> p j h e",
                                                       j=J, h=H))
        sb_Wq = sb_p1[:, 0:2048].rearrange("p (kc n) -> p kc n", kc=4)
        sb_qT = sb_p1[:, 2048:4096].rearrange("p (kc r) -> p kc r", kc=4)
        sb_Wk = sb_p2[:, 0:2048].rearrange("p (kc n) -> p kc n", kc=4)
        sb_kT = sb_p2[:, 2048:4128].rearrange("p (kc r) -> p kc r", kc=4)
        sb_Wv = sb_p2[:, 4128:6176].rearrange("p (kc n) -> p kc n", kc=4)
        sb_vT = sb_p2[:, 6176:8256].rearrange("p (kc r) -> p kc r", kc=4)
        sb_Wo = sb_p3[:, 0:2048].rearrange("p (kc n) -> p kc n", kc=4)
        sb_CA = sb_p3[:, 2048:2560].rearrange("p (j q) -> p j q", j=J)
        sb_CBs = sb_p3[0:8, 2560:2592].rearrange("p (j q) -> p j q", j=J)
        sb_id = sb_p3[:, 2592:2720]
        if with_bias:
            sb_bq = sing.tile([1, DM], BF16)
            nc.sync.dma_start(sb_bq, bq_d[:])
            sb_bk = sing.tile([1, DM], BF16)
            nc.sync.dma_start(sb_bk, bk_d[:])
            sb_bv = sing.tile([1, DM], BF16)
            nc.sync.dma_start(sb_bv, bv_d[:])

        sb_ones_r = sing.tile([1, ROWS], BF16)    # ones row (rhs for bias/Vbg)
        nc.vector.memset(sb_ones_r, 1.0)
        # double-buffered exp(C*S) halo piece; off-band cells stay exp(0)=1
        sb_eBa = sing.tile([8, H * 128], BF16)
        sb_eBb = sing.tile([8, H * 128], BF16)
        sb_eB2 = [sb_eBa, sb_eBb]
        nc.vector.memset(sb_eBa, 1.0)
        nc.vector.memset(sb_eBb, 1.0)

        # persistent intermediates: qhT/khT pack head pair 2i/2i+1 per 128-
        # partition group hp (head 2hp on partitions 0-63, 2hp+1 on 64-127),
        # which makes the d_k=64 score matmuls row-tile into both halves of
        # the PE array.
        sb_qhT = sing.tile([128, 4, ROWS], BF16)
        sb_khT = sing.tile([128, 4, HALO], BF16)
        sb_vh = sing.tile([128, 5, H, 66], BF16)  # row tiles; 66 = 64d+denom+pad (4B-aligned head stride)
        sb_concat = sing.tile([128, J, DM], BF16)
        sb_concatT = sing.tile([128, 4, ROWS], BF16)

        # ---------------- phase 1: projections ----------------
        with tc.tile_pool(name="ppj", bufs=2, space="PSUM") as ppj, \
             tc.tile_pool(name="ppv", bufs=2, space="PSUM") as ppv, \
             tc.tile_pool(name="ppt", bufs=2, space="PSUM") as ppt:
            for hp in range(4):
                ps = ppj.tile([128, ROWS], F32, tag="pj")
                for kc in range(4):
                    nc.tensor.matmul(ps, sb_Wq[:, kc, 128 * hp:128 * hp + 128],
                                     sb_qT[:, kc, :], start=(kc == 0),
                                     stop=(kc == 3 and not with_bias))
                if with_bias:
                    nc.tensor.matmul(ps, sb_bq[0:1, 128 * hp:128 * hp + 128],
                                     sb_ones_r, start=False, stop=True)
                eng = nc.scalar.copy if hp % 2 == 0 else nc.vector.tensor_copy
                eng(sb_qhT[:, hp, :], ps)
            for hp in range(4):
                ps = ppj.tile([128, ROWS], F32, tag="pj")
                pst = ppt.tile([128, 8], F32, tag="pt")
                for kc in range(4):
                    nc.tensor.matmul(ps, sb_Wk[:, kc, 128 * hp:128 * hp + 128],
                                     sb_kT[:, kc, 0:512], start=(kc == 0),
                                     stop=(kc == 3 and not with_bias))
                    nc.tensor.matmul(pst, sb_Wk[:, kc, 128 * hp:128 * hp + 128],
                                     sb_kT[:, kc, 512:HALO], start=(kc == 0),
                                     stop=(kc == 3 and not with_bias))
                if with_bias:
                    nc.tensor.matmul(ps, sb_bk[0:1, 128 * hp:128 * hp + 128],
                                     sb_ones_r, start=False, stop=True)
                    nc.tensor.matmul(pst, sb_bk[0:1, 128 * hp:128 * hp + 128],
                                     sb_ones_r[0:1, 0:8], start=False, stop=True)
                eng = nc.scalar.copy if hp % 2 == 0 else nc.vector.tensor_copy
                eng(sb_khT[:, hp, 0:512], ps)
                eng2 = nc.vector.tensor_copy if hp % 2 == 0 else nc.scalar.copy
                eng2(sb_khT[:, hp, 512:HALO], pst)
            # vh[rows(+halo shift), dout] in 65-strided head blocks
            for rt in range(5):
                nr = 128 if rt < 4 else 8
                ps = ppv.tile([128, DM], F32, tag="pv")
                for kc in range(4):
                    nc.tensor.matmul(ps[0:nr, :],
                                     sb_vT[:, kc, 128 * rt:128 * rt + nr],
                                     sb_Wv[:, kc, :], start=(kc == 0),
                                     stop=(kc == 3 and not with_bias))
                if with_bias:
                    nc.tensor.matmul(ps[0:nr, :], sb_ones_r[0:1, 0:nr], sb_bv,
                                     start=False, stop=True)
                eng = nc.scalar.copy if rt % 2 == 0 else nc.vector.tensor_copy
                eng(sb_vh[0:nr, rt, :, 0:64],
                    ps[0:nr, :].rearrange("p (h d) -> p h d", h=H))
            nc.vector.memset(sb_vh[:, :, :, 64:65], 1.0)

        # ---------------- phases 2-4, software-pipelined ----------------
        with tc.tile_pool(name="pSA", bufs=1, space="PSUM") as pSA, \
             tc.tile_pool(name="pSB", bufs=1, space="PSUM") as pSB, \
             tc.tile_pool(name="pN", bufs=2, space="PSUM") as pN, \
             tc.tile_pool(name="pT", bufs=1, space="PSUM") as pT, \
             tc.tile_pool(name="pF", bufs=1, space="PSUM") as pF, \
             tc.tile_pool(name="att", bufs=2) as att, \
             tc.tile_pool(name="fout", bufs=2) as fout:

            eAs = [None] * J
            eBs = [None] * J

            def scores(j):
                """S = kh.T qh over the 136-wide window; E = exp(C*S)."""
                psa = pSA.tile([128, H * 128], F32, tag="sa")
                psb = pSB.tile([8, 2, 512], F32, tag="sb")
                # slot s = 4*par + hp: the concurrent row-tiled pair (par 0/1
                # run simultaneously in disjoint PE row groups) must drain to
                # DIFFERENT PSUM banks - same-bank concurrent drain faults HW
                for hp in range(4):
                    for par in range(2):
                        b0, s = 64 * par, 4 * par + hp
                        nc.tensor.matmul(
                            psa[:, 128 * s:128 * s + 128],
                            sb_khT[b0:b0 + 64, hp, 128 * j:128 * j + 128],
                            sb_qhT[b0:b0 + 64, hp, 128 * j:128 * j + 128],
                            start=True, stop=True)
                        nc.tensor.matmul(
                            psb[:, par, 8 * hp:8 * hp + 8],
                            sb_khT[b0:b0 + 64, hp, 128 * j + 128:128 * j + 136],
                            sb_qhT[b0:b0 + 64, hp, 128 * j + 120:128 * j + 128],
                            start=True, stop=True)
                ca = sb_CA[:, j, :]
                ca_b = bass.AP(tensor=ca.tensor, offset=ca.offset,
                               ap=[list(ca.ap[0]), [0, H], list(ca.ap[1])])
                psa_v = psa[:].rearrange("p (h q) -> p h q", h=H)
                nc.vector.tensor_mul(psa_v, psa_v, ca_b)
                eA = att.tile([128, H * 128], BF16, tag="eA")
                nc.scalar.activation(eA, psa, EXP)
                eB = sb_eB2[j % 2]
                cb = sb_CBs[:, j, :]
                cb_b = bass.AP(tensor=cb.tensor, offset=cb.offset,
                               ap=[list(cb.ap[0]), [0, 4], list(cb.ap[1])])
                for par in range(2):
                    psb_v = psb[:, par, 0:32].rearrange("p (hp q) -> p hp q",
                                                        hp=4)
                    nc.vector.tensor_mul(psb_v, psb_v, cb_b)
                    eB_v = eB[:, 512 * par:512 * par + 512].rearrange(
                        "p (hp q) -> p hp q", hp=4)[:, :, 120:128]
                    nc.scalar.activation(eB_v, psb_v, EXP)
                eAs[j] = eA
                eBs[j] = eB

            def numer_div(j):
                eA, eB = eAs[j], eBs[j]
                pn0 = pN.tile([128, 4, 65], F32, tag="n")
                pn1 = pN.tile([128, 4, 65], F32, tag="n")
                # background term: one K=1 matmul covers 4 heads (rhs strides
                # across the 66-wide vs65 slots), replacing 8 per-head matmuls
                nc.tensor.matmul(pn0[:, :, :], sb_ones_r[0:1, 0:128],
                                 sb_vs65[0:1, j, 0:4, 0:65],
                                 start=True, stop=False, skip_group_check=True)
                nc.tensor.matmul(pn1[:, :, :], sb_ones_r[0:1, 0:128],
                                 sb_vs65[0:1, j, 4:8, 0:65],
                                 start=True, stop=False, skip_group_check=True)
                for h in range(H):
                    pn = (pn0 if h < 4 else pn1)[:, h % 4, :]
                    nc.tensor.matmul(pn, eA[:, 128 * h:128 * h + 128],
                                     sb_vh[:, j, h, 0:65], start=False,
                                     stop=False, skip_group_check=True)
                    nc.tensor.matmul(pn, eB[:, 128 * h:128 * h + 128],
                                     sb_vh[0:8, j + 1, h, 0:65], start=False,
                                     stop=True, skip_group_check=True)
                r = att.tile([128, H], F32, tag="r")
                nc.vector.reciprocal(r[:, 0:4], pn0[:, :, 64])
                nc.vector.reciprocal(r[:, 4:8], pn1[:, :, 64])
                for pn, ho in ((pn0, 0), (pn1, 4)):
                    rs = r[:, ho:ho + 4]
                    r_b = bass.AP(tensor=rs.tensor, offset=rs.offset,
                                  ap=[list(rs.ap[0]), list(rs.ap[1]), [0, 64]])
                    outv = sb_concat[:, j, 64 * ho:64 * ho + 256]
                    nc.vector.scalar_tensor_tensor(
                        outv.rearrange("p (h d) -> p h d", h=4),
                        pn[:, :, 0:64], 1.0, r_b, op0=MULT, op1=MULT)
                # transpose this q-block of concat on the PE
                if upto < 3:
                    return
                for dc in range(4):
                    pt = pT.tile([128, 128], BF16, tag="t")
                    nc.tensor.transpose(pt, sb_concat[:, j, 128 * dc:128 * dc + 128],
                                        sb_id)
                    eng = nc.scalar.copy if dc % 2 == 0 else nc.vector.tensor_copy
                    eng(sb_concatT[:, dc, 128 * j:128 * j + 128], pt)

            def outproj(rc):
                if upto < 4:
                    so = fout.tile([128, DM], F32, tag="fo")
                    src = sb_concatT if upto == 3 else sb_concat
                    nc.vector.tensor_copy(so, src[:, rc, 0:DM])
                    nc.sync.dma_start(out_d[128 * rc:128 * rc + 128, :], so)
                    return
                pf = pF.tile([128, DM], F32, tag="f")
                for dc in range(4):
                    nc.tensor.matmul(pf, sb_concatT[:, dc, 128 * rc:128 * rc + 128],
                                     sb_Wo[:, dc, :], start=(dc == 0),
                                     stop=(dc == 3))
                so = fout.tile([128, DM], F32, tag="fo")
                if rc == 3:
                    nc.vector.tensor_copy(so[:, 0:256], pf[:, 0:256])
                    nc.sync.dma_start(out_d[128 * rc:128 * rc + 128, 0:256],
                                      so[:, 0:256])
                    nc.scalar.copy(so[:, 256:512], pf[:, 256:512])
                    nc.sync.dma_start(out_d[128 * rc:128 * rc + 128, 256:512],
                                      so[:, 256:512])
                else:
                    eng = nc.scalar.copy if rc % 2 == 0 else nc.vector.tensor_copy
                    eng(so, pf)
                    nc.sync.dma_start(out_d[128 * rc:128 * rc + 128, :], so)

            if upto >= 2:
                import os
                if os.environ.get("BASS_SEQ", "0") == "1":
                    for j in range(J):
                        scores(j)
                        numer_div(j)
                    for rc in range(4):
                        outproj(rc)
                else:
                    scores(0)
                    scores(1)
                    numer_div(0)
                    scores(2)
                    numer_div(1)
                    scores(3)
                    numer_div(2)
                    outproj(0)
                    numer_div(3)
                    outproj(1)
                    outproj(2)
                    outproj(3)
            else:
                for rc in range(4):
                    so = fout.tile([128, 256], F32, tag="dbg")
                    nc.vector.tensor_copy(so, sb_vh[:, rc, 0:4, 0:64])
                    nc.sync.dma_start(out_d[128 * rc:128 * rc + 128, 0:256], so)

    if not nc.is_finalized():
        nc.finalize()
    return nc


_PROG_CACHE = {}


def _get_program(with_bias):
    import os
    upto = int(os.environ.get("BASS_KERNEL_UPTO", "4"))
    key = (bool(with_bias), upto)
    if key not in _PROG_CACHE:
        _PROG_CACHE[key] = _build_program(with_bias, upto)
    return _PROG_CACHE[key]


# ----------------------------------------------------------------------------
# entry point
# ----------------------------------------------------------------------------

def prep_in_maps(q, k, v, Wq, bq, Wk, bk, Wv, bv, Wo, bo, **_unused):
    """Builds per-core input maps + the traced program; returns (in_maps, nc)."""
    q = np.asarray(q, np.float32)
    k = np.asarray(k, np.float32)
    v = np.asarray(v, np.float32)
    Wq_b = np.ascontiguousarray(Wq, np.float32).astype(NPBF)
    Wk_b = np.ascontiguousarray(Wk, np.float32).astype(NPBF)
    # slot permutation: scores slot s = 4*par+hp holds head 2*hp+par, so
    # permute Wv cols / Wo rows to slot order once on the host
    PERM = [0, 2, 4, 6, 1, 3, 5, 7]
    pcols = np.concatenate([np.arange(64 * p, 64 * p + 64) for p in PERM])
    Wv32 = np.ascontiguousarray(np.asarray(Wv, np.float32)[:, pcols])
    Wv_b = Wv32.astype(NPBF)
    Wo_b = np.ascontiguousarray(np.asarray(Wo, np.float32)[pcols, :]).astype(NPBF)
    bq = np.asarray(bq, np.float32).reshape(-1)
    bk = np.asarray(bk, np.float32).reshape(-1)
    bv = np.asarray(bv, np.float32).reshape(-1)[
        np.concatenate([np.arange(64 * p, 64 * p + 64)
                        for p in [0, 2, 4, 6, 1, 3, 5, 7]])]
    with_bias = bool(np.any(bq) or np.any(bk) or np.any(bv))
    nc = _get_program(with_bias)

    C = _band_count() / np.float32(math.sqrt(DK))
    vsum = v.sum(axis=1)  # [2, 512]

    in_maps = []
    for c in range(8):
        b, R0 = c // 4, ROWS * (c % 4)
        qT = np.ascontiguousarray(q[b, R0:R0 + ROWS, :].T.astype(NPBF))
        kT = np.zeros((DM, HALO), NPBF)
        vT = np.zeros((DM, HALO), NPBF)
        g0 = R0 - 4
        s0, s1 = max(g0, 0), min(R0 + ROWS + 4, SEQ)
        kT[:, s0 - g0:s1 - g0] = k[b, s0:s1, :].T.astype(NPBF)
        vT[:, s0 - g0:s1 - g0] = v[b, s0:s1, :].T.astype(NPBF)
        CA, CB = _c_tiles(R0, C)
        CBs = np.ascontiguousarray(CB[:, :, 120:128])
        assert not CB[:, :, :120].any()
        # vs65[j,h,:] = [background-v sum for window j projected by Wv, count]
        vs65 = np.zeros((J, H, 66), np.float32)
        for j in range(J):
            w0, w1 = max(R0 + 128 * j - 4, 0), min(R0 + 128 * j + 132, SEQ)
            vbg = vsum[b] - v[b, w0:w1, :].sum(axis=0)
            vs65[j, :, 0:64] = (vbg @ Wv32 + BGCNT * bv).reshape(H, 64)
            vs65[j, :, 64] = BGCNT
        def pkc(x):  # [512, cols] -> [128, 4*cols] in (p, kc, cols) layout
            c = x.shape[1]
            return x.reshape(4, 128, c).transpose(1, 0, 2).reshape(128, 4 * c)

        p1 = np.concatenate([pkc(Wq_b), pkc(qT)], axis=1)
        p2 = np.concatenate([pkc(Wk_b), pkc(kT), pkc(Wv_b), pkc(vT)], axis=1)
        cbs_blk = np.zeros((128, 32), NPBF)
        cbs_blk[0:8, :] = CBs.astype(NPBF).transpose(1, 0, 2).reshape(8, 32)
        p3 = np.concatenate([pkc(Wo_b),
                             CA.astype(NPBF).transpose(1, 0, 2).reshape(128, 512),
                             cbs_blk, np.eye(128, dtype=NPBF)], axis=1)
        m = {"p1": np.ascontiguousarray(p1), "p2": np.ascontiguousarray(p2),
             "p3": np.ascontiguousarray(p3),
             "vs65": np.ascontiguousarray(vs65.reshape(1, -1).astype(NPBF))}
        if with_bias:
            m["bq"] = bq[None, :].astype(NPBF)
            m["bk"] = bk[None, :].astype(NPBF)
            m["bv"] = bv[None, :].astype(NPBF)
        in_maps.append(m)
    return in_maps, nc


def kernel(q, k, v, Wq, bq, Wk, bk, Wv, bv, Wo, bo, **_unused):
    bo = np.asarray(bo, np.float32).reshape(-1)
    in_maps, nc = prep_in_maps(q, k, v, Wq, bq, Wk, bk, Wv, bv, Wo, bo)
    res = run_bass_kernel_spmd(nc, in_maps, core_ids=list(range(8)))
    out = np.empty((2, SEQ, DM), np.float32)
    for c in range(8):
        b, R0 = c // 4, ROWS * (c % 4)
        out[b, R0:R0 + ROWS, :] = res.results[c]["out"]
    if np.any(bo):
        out += bo
    return out


if __name__ == "__main__":
    rng = np.random.default_rng(0)
    s = 1.0 / math.sqrt(DM)
    inp = dict(
        q=rng.standard_normal((2, SEQ, DM)).astype(np.float32),
        k=rng.standard_normal((2, SEQ, DM)).astype(np.float32),
        v=rng.standard_normal((2, SEQ, DM)).astype(np.float32),
        Wq=(rng.standard_normal((DM, DM)) * s).astype(np.float32),
        bq=np.zeros(DM, np.float32),
        Wk=(rng.standard_normal((DM, DM)) * s).astype(np.float32),
        bk=np.zeros(DM, np.float32),
        Wv=(rng.standard_normal((DM, DM)) * s).astype(np.float32),
        bv=np.zeros(DM, np.float32),
        Wo=(rng.standard_normal((DM, DM)) * s).astype(np.float32),
        bo=np.zeros(DM, np.float32),
    )
    out = kernel(**inp)
    print("kernel ran, out shape", out.shape, "mean", np.abs(out).mean())
